# revision 3
# baseline (speedup 1.0000x reference)
"""GRU encoder (nn_Encoder_26087631356042) Bass/Trainium2 kernel.

Strategy: data-parallel over batch (B=128 -> 16 per core, 8 cores, no
collectives). Per core, a fused kernel: the input projection GEMM
(x @ W_ih.T) is computed 32 timesteps at a time inside the sequential
GRU time loop, entirely in feature-major "packed" layout
(feature f -> (block m = f//128, partition p = f%128)), so all gate
elementwise ops run with 128 active partitions and tiny free dims.

The recurrent matmul keeps W_hh.T stationary (bf16 hi+lo split) and
streams the hidden state (bf16 hi+lo split) as the moving operand,
accumulating exactly in fp32 PSUM; biases enter through a rank-1
"bias matmul" that also serves as the accumulation-group opener.
"""

import os
import numpy as np
import ml_dtypes
from contextlib import ExitStack

import concourse.bass as bass
import concourse.bacc as bacc
import concourse.tile as tile
import concourse.mybir as mybir
from concourse.bass_utils import run_bass_kernel_spmd

F32 = mybir.dt.float32
BF16 = mybir.dt.bfloat16
AF = mybir.ActivationFunctionType

B, T, X, H = 128, 2048, 128, 256
G = 3 * H          # 768 gate features
NBLK = G // 128    # 6 feature blocks
NCORES = 8
BL = B // NCORES   # 16 batch rows per core
CH = 64            # timesteps per For_i body
P = 128

bf16 = ml_dtypes.bfloat16


def _split_hi_lo(a32: np.ndarray):
    hi = a32.astype(bf16)
    lo = (a32 - hi.astype(np.float32)).astype(bf16)
    return hi, lo


def _build_program(t_steps: int, reps: int = 1, nogates: bool = False,
                   single: bool = False, nophase1: bool = False,
                   f32r_rhs: bool = False, allr: bool = False,
                   coltile: bool = False, unroll: bool = False):
    """Emit the per-core program (same program on all cores; data differs).

    reps > 1 wraps the whole computation in an outer repeat loop (state
    carries over between reps — outputs are only timing-valid).
    nogates/single/nophase1 are timing-ablation variants."""
    nchunks = t_steps // CH
    nc = bacc.Bacc(
        "TRN2", target_bir_lowering=False, debug=False, num_devices=NCORES
    )

    # DRAM I/O
    d_xin_hi = nc.dram_tensor("xin_hi", [P, t_steps * BL], BF16, kind="ExternalInput")
    d_xin_lo = nc.dram_tensor("xin_lo", [P, t_steps * BL], BF16, kind="ExternalInput")
    d_whh_hi = nc.dram_tensor("whh_hi", [P, 2 * G], BF16, kind="ExternalInput")
    d_whh_lo = nc.dram_tensor("whh_lo", [P, 2 * G], BF16, kind="ExternalInput")
    d_wih_hi = nc.dram_tensor("wih_hi", [P, G], BF16, kind="ExternalInput")
    d_wih_lo = nc.dram_tensor("wih_lo", [P, G], BF16, kind="ExternalInput")
    d_biasmat = nc.dram_tensor("biasmat", [P, P], BF16, kind="ExternalInput")
    d_sel = nc.dram_tensor("sel", [P, NBLK * BL], BF16, kind="ExternalInput")
    d_bihn = nc.dram_tensor("bihn", [P, 2], F32, kind="ExternalInput")
    if allr:
        d_whh_f = nc.dram_tensor("whh_f", [P, 2 * G], F32, kind="ExternalInput")
        d_biasmat_f = nc.dram_tensor("biasmat_f", [P, P], F32, kind="ExternalInput")
        d_sel_f = nc.dram_tensor("sel_f", [P, NBLK * BL], F32, kind="ExternalInput")
    d_out = nc.dram_tensor("hout", [P, 2 * BL], F32, kind="ExternalOutput")

    with tile.TileContext(nc) as tc, ExitStack() as ctx:
        cpool = ctx.enter_context(tc.tile_pool(name="const", bufs=1))
        state = ctx.enter_context(tc.tile_pool(name="state", bufs=1))
        xpp = ctx.enter_context(tc.tile_pool(name="xp", bufs=1))
        xinp = ctx.enter_context(tc.tile_pool(name="xin", bufs=2))
        gsb = ctx.enter_context(tc.tile_pool(name="gates", bufs=2))
        php = ctx.enter_context(tc.tile_pool(name="php", bufs=2, space="PSUM"))
        phpn = ctx.enter_context(tc.tile_pool(name="phpn", bufs=2, space="PSUM"))
        pscr = ctx.enter_context(tc.tile_pool(name="pscr", bufs=2, space="PSUM"))
        px = ctx.enter_context(tc.tile_pool(name="px", bufs=2, space="PSUM"))

        # Constants -> SBUF
        whh_hi = cpool.tile([P, 2 * G], BF16, tag="whh_hi")
        whh_lo = cpool.tile([P, 2 * G], BF16, tag="whh_lo")
        wih_hi = cpool.tile([P, G], BF16, tag="wih_hi")
        wih_lo = cpool.tile([P, G], BF16, tag="wih_lo")
        biasmat = cpool.tile([P, P], BF16, tag="biasmat")
        sel = cpool.tile([P, NBLK * BL], BF16, tag="sel")
        bihn = cpool.tile([P, 2], F32, tag="bihn")
        loads = [
            (whh_hi, d_whh_hi), (whh_lo, d_whh_lo),
            (wih_hi, d_wih_hi), (wih_lo, d_wih_lo),
            (biasmat, d_biasmat), (sel, d_sel), (bihn, d_bihn),
        ]
        if allr:
            whh_f = cpool.tile([P, 2 * G], F32, tag="whh_f")
            biasmat_f = cpool.tile([P, P], F32, tag="biasmat_f")
            sel_f = cpool.tile([P, NBLK * BL], F32, tag="sel_f")
            loads += [(whh_f, d_whh_f), (biasmat_f, d_biasmat_f), (sel_f, d_sel_f)]
            whh_r = whh_f.bitcast(mybir.dt.float32r)
            biasmat_r = biasmat_f.bitcast(mybir.dt.float32r)
            sel_r = sel_f.bitcast(mybir.dt.float32r)
        for dst, src in loads:
            nc.sync.dma_start(dst[:], src.ap()[:])

        # Hidden state (feature-major packed): [128, 2 k-blocks, 16 batch]
        # Ping-pong pairs; CH is even so every body starts and ends on idx 0.
        hT = [state.tile([P, 2, BL], F32, name=f"hT{j}", tag=f"hT{j}") for j in range(2)]
        hTr = [t.bitcast(mybir.dt.float32r) for t in hT]
        hhi = [state.tile([P, 2, BL], BF16, name=f"hhi{j}", tag=f"hhi{j}") for j in range(2)]
        hlo = [state.tile([P, 2, BL], BF16, name=f"hlo{j}", tag=f"hlo{j}") for j in range(2)]
        for t_ in (hT[0], hhi[0], hlo[0]):
            nc.gpsimd.memset(t_[:], 0)

        # xp slab for one chunk: [128, 6 blocks, CH*BL cols] fp32
        xp = xpp.tile([P, NBLK, CH * BL], F32, tag="xp")
        if nophase1:
            nc.gpsimd.memset(xp[:], 0)

        from contextlib import contextmanager

        @contextmanager
        def _chunk_iter():
            if unroll:
                yield list(range(nchunks))
            else:
                with tc.For_i(
                    0, nchunks,
                    hint_engines=(mybir.EngineType.PE, mybir.EngineType.DVE),
                ) as ci:
                    yield [ci]

        def emit_time_loop():
          with _chunk_iter() as cis:
           for ci in cis:
            # ---- Phase 1: xp = Wih @ x for CH steps (feature-major) ----
            xh = xinp.tile([P, CH * BL], BF16, tag="xh")
            xl = xinp.tile([P, CH * BL], BF16, tag="xl")
            nc.sync.dma_start(xh[:], d_xin_hi.ap()[:, bass.ts(ci, CH * BL)])
            nc.sync.dma_start(xl[:], d_xin_lo.ap()[:, bass.ts(ci, CH * BL)])
            for m in range(NBLK if not nophase1 else 0):
                for hf in range(CH * BL // 512):
                    pxm = px.tile([P, 512], F32, tag="pxm")
                    wsl = slice(128 * m, 128 * (m + 1))
                    xsl = slice(512 * hf, 512 * (hf + 1))
                    nc.tensor.matmul(pxm[:], wih_hi[:, wsl], xh[:, xsl],
                                     start=True, stop=False)
                    nc.tensor.matmul(pxm[:], wih_hi[:, wsl], xl[:, xsl],
                                     start=False, stop=False)
                    nc.tensor.matmul(pxm[:], wih_lo[:, wsl], xh[:, xsl],
                                     start=False, stop=True)
                    if m < 4:
                        nc.vector.tensor_copy(xp[:, m, xsl], pxm[:])
                    else:
                        # fold b_ih (n-gate part) in during evacuation
                        nc.scalar.activation(
                            xp[:, m, xsl], pxm[:], AF.Identity,
                            bias=bihn[:, m - 4: m - 3],
                        )

            # ---- Recurrence over CH steps ----
            for s in range(CH):
                cur, nxt = s % 2, (s + 1) % 2
                # split psum tiles: rz completes first so the sigmoid path
                # overlaps the n-block matmuls (deps are tile-granular)
                hprz = php.tile([P, 4, BL], F32, tag="hprz")
                hpn = phpn.tile([P, 2, BL], F32, tag="hpn")
                # bias matmuls open the accumulation groups (shared lhsT)
                bm = biasmat_r if allr else biasmat
                sl = sel_r if allr else sel
                nc.tensor.matmul(hprz.rearrange("p a b -> p (a b)"),
                                 bm[:], sl[:, 0:4 * BL],
                                 start=True, stop=False)
                nc.tensor.matmul(hpn.rearrange("p a b -> p (a b)"),
                                 bm[:], sl[:, 4 * BL:],
                                 start=True, stop=False)

                def emit_mms(ms, tgt, off):
                    for mi, m in enumerate(ms):
                        for k in range(2):
                            rh = hhi[cur][:, k, :]
                            rl = hlo[cur][:, k, :]
                            wsl = slice(G * k + 128 * m, G * k + 128 * (m + 1))
                            last = (k == 1 and mi == len(ms) - 1)
                            o = tgt[:, m - off, :]
                            if allr:
                                rf = hTr[cur][:, k, :]
                                nc.tensor.matmul(o, whh_r[:, wsl], rf,
                                                 start=False, stop=last)
                                continue
                            if f32r_rhs:
                                # exact h streamed as fp32r against bf16 weights
                                rf = hTr[cur][:, k, :]
                                nc.tensor.matmul(o, whh_hi[:, wsl], rf,
                                                 start=False, stop=False)
                                nc.tensor.matmul(o, whh_lo[:, wsl], rf,
                                                 start=False, stop=last)
                                continue
                            if single:
                                nc.tensor.matmul(o, whh_hi[:, wsl], rh,
                                                 start=False, stop=last)
                                continue
                            if coltile:
                                # [128,32] weight subtiles -> 4 col-groups of
                                # the PE array load + compute concurrently
                                base = G * k + 128 * m
                                for ti, (w, r) in enumerate(
                                    ((whh_hi, rh), (whh_hi, rl), (whh_lo, rh))
                                ):
                                    for q in range(4):
                                        qs = slice(base + 32 * q,
                                                   base + 32 * (q + 1))
                                        oq = o[32 * q: 32 * (q + 1), :]
                                        nc.tensor.matmul(
                                            oq, w[:, qs], r,
                                            start=False,
                                            stop=(last and ti == 2 and q == 3),
                                            tile_position=(0, 32 * q),
                                        )
                                continue
                            nc.tensor.matmul(o, whh_hi[:, wsl], rh,
                                             start=False, stop=False)
                            nc.tensor.matmul(o, whh_hi[:, wsl], rl,
                                             start=False, stop=False)
                            nc.tensor.matmul(o, whh_lo[:, wsl], rh,
                                             start=False, stop=last)

                emit_mms((0, 1, 2, 3), hprz, 0)
                emit_mms((4, 5), hpn, 4)

                xp_t = xp[:, :, bass.ts(s, BL)]          # [128, 6, 16]
                scr = pscr.tile([P, NBLK, BL], F32, tag="scr")
                rz = gsb.tile([P, 4, BL], F32, tag="rz")
                t1 = gsb.tile([P, 2, BL], F32, tag="t1")
                nsb = gsb.tile([P, 2, BL], F32, tag="nsb")
                zc = gsb.tile([P, 2, BL], F32, tag="zc")
                zh = gsb.tile([P, 2, BL], F32, tag="zh")
                t3 = gsb.tile([P, 2, BL], F32, tag="t3")

                if nogates:
                    # ablation: keep the serial dep chain, drop gate math
                    nc.vector.tensor_copy(hT[nxt][:], hpn[:, 0:2, :])
                    nc.vector.tensor_copy(hhi[nxt][:], hT[nxt][:])
                    nc.vector.tensor_sub(hlo[nxt][:], hT[nxt][:], hhi[nxt][:])
                    continue
                # r,z pre-activations then sigmoid (overlaps n-block MMs)
                nc.vector.tensor_add(scr[:, 0:4, :], xp_t[:, 0:4, :], hprz[:])
                nc.scalar.activation(rz[:], scr[:, 0:4, :], AF.Sigmoid)
                nc.scalar.activation(zc[:], rz[:, 2:4, :], AF.Copy,
                                     scale=-1.0, bias=1.0)
                # off-critical-path: z*h
                nc.vector.tensor_mul(zh[:], rz[:, 2:4, :], hT[cur][:])
                # n = tanh(xn + r*hn)   (b_ih_n already in xp, b_hh_n in hp)
                nc.vector.tensor_mul(t1[:], rz[:, 0:2, :], hpn[:])
                nc.vector.tensor_add(scr[:, 4:6, :], t1[:], xp_t[:, 4:6, :])
                nc.scalar.activation(nsb[:], scr[:, 4:6, :], AF.Tanh)
                # h' = (1-z)*n + z*h ; emit the bf16 hi part FIRST so the
                # next step's Whi@hhi matmuls can start one op earlier
                nc.vector.tensor_mul(t3[:], nsb[:], zc[:])
                nc.vector.tensor_add(hhi[nxt][:], t3[:], zh[:])
                nc.vector.tensor_add(hT[nxt][:], t3[:], zh[:])
                nc.vector.tensor_sub(hlo[nxt][:], hT[nxt][:], hhi[nxt][:])

        if reps > 1:
            with tc.For_i(0, reps, name="rep"):
                emit_time_loop()
        else:
            emit_time_loop()

        nc.sync.dma_start(d_out.ap()[:], hT[0].rearrange("p a b -> p (a b)"))

    nc.compile()
    return nc


_PROGRAM_CACHE: dict = {}


def _get_program(t_steps: int, reps: int = 1):
    key = (t_steps, reps)
    if key not in _PROGRAM_CACHE:
        _PROGRAM_CACHE[key] = _build_program(t_steps, reps)
    return _PROGRAM_CACHE[key]


def _pack_inputs(input, W_ih, W_hh, b_ih, b_hh, t_steps: int):
    """Host-side packing. Returns per-core in_maps."""
    input = np.asarray(input, np.float32)
    W_ih = np.asarray(W_ih, np.float32)
    W_hh = np.asarray(W_hh, np.float32)
    b_ih = np.asarray(b_ih, np.float32)
    b_hh = np.asarray(b_hh, np.float32)

    # weights, feature-major packed (shared by all cores)
    whhT = np.ascontiguousarray(W_hh.T)              # [H, G]
    whh = whhT.reshape(2, P, G).transpose(1, 0, 2).reshape(P, 2 * G)
    whh_hi, whh_lo = _split_hi_lo(np.ascontiguousarray(whh))
    wihT = np.ascontiguousarray(W_ih.T)              # [X, G] = [128, 768]
    wih_hi, wih_lo = _split_hi_lo(wihT)

    # bias matrix: rows 0..5 hi parts, rows 6..11 lo parts; selector picks both
    bias_full = b_hh.copy()
    bias_full[: 2 * H] += b_ih[: 2 * H]              # r,z: b_ih + b_hh; n: b_hh
    bmat32 = np.zeros((P, P), np.float32)
    bvec = bias_full.reshape(NBLK, P)
    bhi = bvec.astype(bf16).astype(np.float32)
    blo = bvec - bhi
    bmat32[0:NBLK, :] = bhi
    bmat32[NBLK: 2 * NBLK, :] = blo
    biasmat = bmat32.astype(bf16)
    selmat = np.zeros((P, NBLK * BL), np.float32)
    for m in range(NBLK):
        selmat[m, BL * m: BL * (m + 1)] = 1.0
        selmat[m + NBLK, BL * m: BL * (m + 1)] = 1.0
    sel = selmat.astype(bf16)
    bihn = np.ascontiguousarray(b_ih[2 * H:].reshape(2, P).T)  # [128, 2]

    shared = dict(
        whh_hi=whh_hi, whh_lo=whh_lo, wih_hi=wih_hi, wih_lo=wih_lo,
        biasmat=biasmat, sel=sel, bihn=bihn,
        whh_f=np.ascontiguousarray(whh), biasmat_f=bmat32, sel_f=selmat,
    )
    in_maps = []
    for c in range(NCORES):
        xs = input[c * BL: (c + 1) * BL, :t_steps, :]     # [16, t, 128]
        xt = np.ascontiguousarray(xs.transpose(2, 1, 0))  # [128, t, 16]
        xt = xt.reshape(P, t_steps * BL)
        xh, xl = _split_hi_lo(xt)
        m = dict(shared)
        m["xin_hi"] = xh
        m["xin_lo"] = xl
        in_maps.append(m)
    return in_maps


def _unpack_output(results):
    out = np.empty((B, H), np.float32)
    for c in range(NCORES):
        o = results[c]["hout"].reshape(P, 2, BL)           # [p, k, b]
        out[c * BL: (c + 1) * BL, :] = o.transpose(2, 1, 0).reshape(BL, H)
    return out


def run(input, W_ih, W_hh, b_ih, b_hh, t_steps: int = T, trace: bool = False):
    nc = _get_program(t_steps)
    in_maps = _pack_inputs(input, W_ih, W_hh, b_ih, b_hh, t_steps)
    res = run_bass_kernel_spmd(
        nc, in_maps, core_ids=list(range(NCORES)), trace=trace
    )
    return _unpack_output(res.results), res


def kernel(input, W_ih, W_hh, b_ih, b_hh):
    out, _ = run(input, W_ih, W_hh, b_ih, b_hh)
    return out


def bench(input, W_ih, W_hh, b_ih, b_hh, reps_hi: int = 5, iters: int = 3):
    """Estimate on-device time: wall(R=reps_hi) - wall(R=1) over cached
    executables, divided by (reps_hi - 1). Returns ns."""
    import time as _time

    in_maps = _pack_inputs(input, W_ih, W_hh, b_ih, b_hh, T)
    nc1 = _get_program(T, 1)
    ncR = _get_program(T, reps_hi)

    def timed(nc):
        best = float("inf")
        for _ in range(iters):
            t0 = _time.perf_counter()
            run_bass_kernel_spmd(nc, in_maps, core_ids=list(range(NCORES)))
            best = min(best, _time.perf_counter() - t0)
        return best

    # warm both executables (compile cache)
    run_bass_kernel_spmd(nc1, in_maps, core_ids=list(range(NCORES)))
    run_bass_kernel_spmd(ncR, in_maps, core_ids=list(range(NCORES)))
    t1 = timed(nc1)
    tR = timed(ncR)
    ns = (tR - t1) / (reps_hi - 1) * 1e9
    print(f"wall R=1: {t1*1e3:.1f} ms   wall R={reps_hi}: {tR*1e3:.1f} ms")
    return ns

